# revision 11
# baseline (speedup 1.0000x reference)
"""Trainium2 Bass kernel for nn_DQNAgent_modify (dense_mlp) — fp8 DoubleRow.

Reference computation:
    q_before = mlp(obs.reshape(bs, -1))                      # raw obs
    pert[b, i, k] = obs_flat[b] - onehot(i) x feat[b, k]     # bs*2N rows
    q_after = mlp(pert / norm)                               # [bs, 2N]
    out = q_after - q_before                                 # [bs, 2N]

Two structural facts drive this version:

1. Layer-0 collapse (as in the f32r baseline): the perturbation touches
   only 4 of the 512 input features, so layer 0 of the big batch is
   z = base - corr with base = (obs/norm) @ W0a computed once per
   sample; both terms are matmuls against host-built operands (sel
   broadcast matrix + block-diagonal S from feat).

2. The output q_after - q_before is norm-dominated by q_before (RMS
   ~1.67 vs q_after's ~0.033), so the big 16384-row batch tolerates
   large relative error: quantizing every big-batch matmul to fp8-e4m3
   measures 3.4e-3 final rel err (gate 2e-2).  That unlocks
   MatmulPerfMode.DoubleRow: K=256 contracted per pass at 0.5
   cycles/output-row — ~4x f32r PE throughput.  Only the 64-row
   q_before mini-MLP stays f32r.

With the PE thus accelerated the bottleneck is PSUM->SBUF relu+cast
evictions, so: psum tiles are [128, 2(chunk), 512] pairs evicted in one
op (per-mt bias stays a legal per-partition scalar), evictions rotate
across ACT/DVE/Pool by a weighted load balancer, the final Wv result is
DMA'd straight from PSUM, and b0a rides for free in the selector slot
(sel row 64 = ones, base row 64 = b0a).  W2b (K=128) uses a DoubleRow
pass whose two slots are the two chunks' h5 with zero-padded weight
variants, so it too runs at 256 cycles.

Sharding: pure data parallel over batch, 64 samples/core on 8 cores;
weights replicated.  Row order on device is r = (g, i_lo, k, b) with
i = 32g + i_lo; the host unpermutes and applies q_after - q_before.
"""

import numpy as np
import ml_dtypes

import concourse.mybir as mybir
import concourse.tile as tile
from concourse import bacc
from concourse.bass_utils import run_bass_kernel_spmd

N_CORES = 8
BS, N, D = 512, 128, 4
BSL = BS // N_CORES        # 64 samples per core
IN = N * D                 # 512 input features
NG = 4                     # i-groups == 128-row blocks of W0a
NT = 8                     # 512-row chunks per group
NCHUNK = NG * NT           # 32 chunks of 512 rows per core
F32 = mybir.dt.float32
F32R = mybir.dt.float32r
F8 = mybir.dt.float8e4
NP_F8 = ml_dtypes.float8_e4m3
DR = mybir.MatmulPerfMode.DoubleRow
RELU = mybir.ActivationFunctionType.Relu
COPY = mybir.ActivationFunctionType.Copy
ADD = mybir.AluOpType.add
MAX = mybir.AluOpType.max

# fp8 DR-packed dense layers: (name, KT2 = K/256, M)
DR_LAYERS = [("W0b", 1, 512), ("W1a", 2, 512), ("W1b", 2, 256),
             ("W2a", 1, 128)]
BIAS_OF = {"W0b": "b0b", "W1a": "b1a", "W1b": "b1b", "W2a": "b2a",
           "W2b": "b2b"}
# f32r weights for the q_before mini-MLP
WSHAPES = [("W0a", IN, 256), ("W0b", 256, 512), ("W1a", 512, 512),
           ("W1b", 512, 256), ("W2a", 256, 128), ("W2b", 128, 256),
           ("Wv", 256, 1)]
QB_LAYERS = [("W0b", 2, 4), ("W1a", 4, 4), ("W1b", 4, 2), ("W2a", 2, 1),
             ("W2b", 1, 2)]
BSHAPES = [("b0a", 2), ("b0b", 4), ("b1a", 4), ("b1b", 2), ("b2a", 1),
           ("b2b", 2)]

_CACHE = {}


def _build():
    nc = bacc.Bacc("TRN2", target_bir_lowering=False, debug=False,
                   num_devices=N_CORES)

    dram = {}
    for name, kd, md in WSHAPES:
        dram[name] = nc.dram_tensor(name, [kd, md], F32R,
                                    kind="ExternalInput").ap()
    for name, kt2, md in DR_LAYERS:
        dram[name + "8"] = nc.dram_tensor(name + "8", [128, kt2 * 2 * md],
                                          F8, kind="ExternalInput").ap()
    # W2b dual-variant: [128, (var 2, slot 2, 256)]
    dram["W2b8"] = nc.dram_tensor("W2b8", [128, 1024], F8,
                                  kind="ExternalInput").ap()
    # [128, (slot 2, var 16, 16)]: variant v has Wv in column v of its
    # 16-wide block, zeros elsewhere; 16-wide blocks keep the DR
    # ldweights slot stride 16B-aligned (s3_lw_dual_fp8_restrictions).
    # Variant v routes chunk (16w+v)'s Wv product to psum partition v,
    # so a 16-chunk window accumulates into one [16, 512] psum bank.
    dram["Wv8"] = nc.dram_tensor("Wv8", [128, 512], F8,
                                 kind="ExternalInput").ap()
    # W0a fp8 blocks for L0: [128, (g 4, mt 2, 128)]
    dram["l0w8"] = nc.dram_tensor("l0w8", [128, 1024], F8,
                                  kind="ExternalInput").ap()
    # zpad8: rows 64..127 of the base slot for all 4 g-packs
    # (row 64 = b0a, rest 0)
    dram["zpad8"] = nc.dram_tensor("zpad8", [64, 1024], F8,
                                   kind="ExternalInput").ap()
    # SS8 [128, (cpair 4, slot 2, 1024)]: per chunk PAIR, slot0 = sel
    # twice (row 64 = ones for b0a), slot1 = S columns of both chunks —
    # so one L0 DR matmul covers the pair with an AP-depth-2 ifmap.
    # On-device it lands in four per-cpair tiles: deps are tracked at
    # tile granularity, so a single tile would stall the first L0
    # matmul on ALL four DMAs.
    dram["SS8"] = nc.dram_tensor("SS8", [128, 8192], F8,
                                 kind="ExternalInput").ap()
    # obsU (raw obs for q_before, f32r)
    dram["bundleR"] = nc.dram_tensor("bundleR", [128, 256], F32R,
                                     kind="ExternalInput").ap()
    # obsS8 [128, (kt 4, 64)]: obs/norm in fp8 — base = obsS8 @ W0a runs
    # as fp8 DR so the 640KB f32r obsS/W0a DMA leaves the critical path
    dram["obsS8"] = nc.dram_tensor("obsS8", [128, 256], F8,
                                   kind="ExternalInput").ap()
    dram["bundleF"] = nc.dram_tensor("bundleF", [128, 15], F32,
                                     kind="ExternalInput").ap()
    qa_dram = nc.dram_tensor("qa", [NCHUNK // 16, 8192], F32,
                             kind="ExternalOutput").ap()
    qb_dram = nc.dram_tensor("qb", [1, BSL], F32, kind="ExternalOutput").ap()

    with tile.TileContext(nc) as tc:
        with (
            tc.tile_pool(name="wpool", bufs=1) as wpool,
            tc.tile_pool(name="cpool", bufs=1) as cpool,
            tc.tile_pool(name="hpool", bufs=3) as hpool,
            tc.tile_pool(name="qpool", bufs=3) as qpool,
            tc.tile_pool(name="ps", bufs=3, space="PSUM") as ps,
        ):
            # ---------------- setup DMAs ----------------
            # Every dma_start costs its issuing engine ~700ns of engine
            # time, so the ACT/DVE eviction engines issue NONE; sync and
            # gpsimd queues are ordered by first use.
            obsS8 = cpool.tile([128, 4, 64], F8, name="obsS8")
            nc.sync.dma_start(obsS8.rearrange("p k b -> p (k b)"),
                              dram["obsS8"])
            w0a8 = cpool.tile([128, 4, 2, 128], F8, name="w0a8")
            nc.sync.dma_start(w0a8.rearrange("p g mt m -> p (g mt m)"),
                              dram["l0w8"])

            # l0pack: [128, g 4, mt 2, slot 2, 128]; slot0 = base (+b0a
            # row), slot1 = W0a fp8 block — one tile, two DMA triggers
            l0pack_t = cpool.tile([128, 4, 2, 2, 128], F8, name="l0pack")
            nc.gpsimd.dma_start(
                l0pack_t[:, :, :, 1, :],
                dram["l0w8"].rearrange("p (g mt m) -> p g mt m", g=4, mt=2))
            nc.gpsimd.dma_start(
                l0pack_t[64:128, :, :, 0, :],
                dram["zpad8"].rearrange("p (g mt m) -> p g mt m", g=4, mt=2))
            l0pack = [l0pack_t[:, g, :, :, :] for g in range(4)]

            # SS8 as four per-cpair tiles: cp0/cp1 front-loaded (needed
            # by the two pre-loop emit_l0 calls), cp2/cp3 deferred
            # behind the fp8 weights
            ss8c = [cpool.tile([128, 2, 1024], F8, name=f"ss8_{cp}")
                    for cp in range(4)]
            for cp in (0, 1):
                nc.gpsimd.dma_start(
                    ss8c[cp].rearrange("p s n -> p (s n)"),
                    dram["SS8"][:, 2048 * cp:2048 * (cp + 1)])

            # fp8 DR weights, spread across queues in first-use order
            w8 = {}
            w8_eng = {"W0b": nc.sync, "W1a": nc.gpsimd, "W1b": nc.sync,
                      "W2a": nc.gpsimd}
            for name, kt2, md in DR_LAYERS:
                t = wpool.tile([128, kt2, 2, md], F8, name=f"w8_{name}")
                w8_eng[name].dma_start(
                    t.rearrange("p a s m -> p (a s m)"), dram[name + "8"])
                w8[name] = t
            w8["W2b"] = wpool.tile([128, 2, 2, 256], F8, name="w8_W2b")
            nc.sync.dma_start(
                w8["W2b"].rearrange("p v s m -> p (v s m)"), dram["W2b8"])
            w8["Wv"] = wpool.tile([128, 2, 16, 16], F8, name="w8_Wv")
            nc.gpsimd.dma_start(
                w8["Wv"].rearrange("p s v m -> p (s v m)"), dram["Wv8"])
            for cp, eng in ((2, nc.sync), (3, nc.gpsimd)):
                eng.dma_start(ss8c[cp].rearrange("p s n -> p (s n)"),
                              dram["SS8"][:, 2048 * cp:2048 * (cp + 1)])

            bundleF = cpool.tile([128, 15], F32, name="bundleF")
            nc.sync.dma_start(bundleF[:, :], dram["bundleF"][:, :])

            # f32r weights for q_before: 3.4 MB but not needed until the
            # first qb stage (~25 us in) — queued after all fp8 operands
            w_r = {}
            bundleR = cpool.tile([128, 256], F32R, name="bundleR")
            nc.gpsimd.dma_start(bundleR[:, :], dram["bundleR"][:, :])
            obsU_r = bundleR.rearrange("p (k b) -> p k b", k=4)
            w_r["W0a"] = wpool.tile([128, 4, 256], F32R, name="wr_W0a")
            for k in range(4):
                eng = (nc.sync, nc.gpsimd)[k % 2]
                eng.dma_start(w_r["W0a"][:, k, :],
                              dram["W0a"][128 * k:128 * (k + 1), :])
            for idx, (name, kd, md) in enumerate(WSHAPES[1:]):
                kt = kd // 128
                wr = wpool.tile([128, kt, md], F32R, name=f"wr_{name}")
                eng = (nc.gpsimd, nc.sync)[idx % 2]
                eng.dma_start(
                    wr[:, :, :],
                    dram[name].rearrange("(k p) m -> p k m", p=128))
                w_r[name] = wr

            b_off = {}
            off = 0
            for name, ntc in BSHAPES:
                b_off[name] = off
                off += ntc
            b_sb = {name: bundleF[:, b_off[name]:b_off[name] + ntc]
                    for name, ntc in BSHAPES}

            # ---- base = (obs/norm) @ W0a in fp8 DR (64 rows), evicted
            # as fp8 into slot 0 of all four l0pack tiles
            pbase = ps.tile([BSL, 256], F32, name="ps_base", tag="ps")
            for j in range(2):
                nc.tensor.matmul(
                    pbase[:, :], obsS8[:, 2 * j:2 * j + 2, :],
                    w0a8[:, 2 * j:2 * j + 2, :, :]
                    .rearrange("p k mt m -> p k (mt m)"),
                    start=(j == 0), stop=(j == 1), perf_mode=DR)
            for g in range(4):
                eng = nc.scalar if g % 2 == 0 else nc.vector
                if g % 2 == 0:
                    eng.activation(
                        l0pack[g][0:BSL, :, 0, :],
                        pbase[:, :].rearrange("b (mt m) -> b mt m", mt=2),
                        COPY)
                else:
                    eng.tensor_copy(
                        l0pack[g][0:BSL, :, 0, :],
                        pbase[:, :].rearrange("b (mt m) -> b mt m", mt=2))

            # ------------- weighted eviction scheduler -------------
            # approximate per-op engine costs (ns) incl. fixed overhead
            # GPSIMD cannot access PSUM on TRN2 -> ACT + DVE only
            ev_load = {"act": 0.0, "dve": 0.0}

            def _pick(els):
                costs = {"act": els * 0.83 + 206,
                         "dve": els * 1.04 + 137}
                eng = min(ev_load, key=lambda e: ev_load[e] + costs[e])
                ev_load[eng] += costs[eng]
                return eng

            def evict(out_ap, psum_ap, bias_ap, els):
                b = bias_ap if bias_ap is not None else 0.0
                if _pick(els) == "act":
                    nc.scalar.activation(out_ap, psum_ap, RELU, bias=b)
                else:
                    nc.vector.tensor_scalar(out_ap, psum_ap, b, 0.0, ADD, MAX)

            def evict_copy(out_ap, psum_ap, els):
                if _pick(els) == "act":
                    nc.scalar.activation(out_ap, psum_ap, COPY)
                else:
                    nc.vector.tensor_copy(out_ap, psum_ap)

            # ------- q_before mini-MLP (f32r), one layer per pair -------
            # each stage's evictions get a full pair (~9 us) to complete
            # before the next stage's matmuls need them, so the in-order
            # PE queue never blocks on the ACT/DVE queues.
            qb_state = {}

            def qb_stage0():
                hq = []
                for mt in range(2):
                    pb = ps.tile([128, BSL], F32, name=f"ps_qb0_{mt}",
                                 tag="ps")
                    for kt in range(4):
                        nc.tensor.matmul(
                            pb[:, :],
                            w_r["W0a"][:, kt, 128 * mt:128 * (mt + 1)],
                            obsU_r[:, kt, :],
                            start=(kt == 0), stop=(kt == 3))
                    h = qpool.tile([128, BSL], F32R, name=f"hq0_{mt}",
                                   tag=f"hq_{mt}")
                    evict(h[:, :], pb[:, :], b_sb["b0a"][:, mt:mt + 1], BSL)
                    hq.append(h)
                qb_state["hq"] = hq

            def qb_stage_mid(li):
                wname, ktn, mtn = QB_LAYERS[li]
                hq = qb_state.pop("hq")
                nxt = []
                for mt in range(mtn):
                    pb = ps.tile([128, BSL], F32, name=f"ps_qb{li+1}_{mt}",
                                 tag="ps")
                    for kt in range(ktn):
                        nc.tensor.matmul(
                            pb[:, :],
                            w_r[wname][:, kt, 128 * mt:128 * (mt + 1)],
                            hq[kt][:, :],
                            start=(kt == 0), stop=(kt == ktn - 1))
                    h = qpool.tile([128, BSL], F32R, name=f"hq{li+1}_{mt}",
                                   tag=f"hq_{mt}")
                    evict(h[:, :], pb[:, :],
                          b_sb[BIAS_OF[wname]][:, mt:mt + 1], BSL)
                    nxt.append(h)
                qb_state["hq"] = nxt

            def qb_stage_out():
                hq = qb_state.pop("hq")
                pqb = ps.tile([1, BSL], F32, name="ps_qb_out", tag="ps")
                for kt in range(2):
                    nc.tensor.matmul(pqb[:, :], w_r["Wv"][:, kt, :],
                                     hq[kt][:, :],
                                     start=(kt == 0), stop=(kt == 1))
                qb_sb = qpool.tile([1, BSL], F32, name="qb_sb")
                evict_copy(qb_sb[:, :], pqb[:, :], BSL)
                nc.sync.dma_start(qb_dram[:, :], qb_sb[:, :])

            qb_stages = ([qb_stage0]
                         + [lambda li=i: qb_stage_mid(li) for i in range(5)]
                         + [qb_stage_out])

            # ---------------- big-batch layers (fp8 DR) ----------------
            def emit_l0(ci0):
                """L0 for a chunk pair -> h1pair [128, mtK 2, chunk 2, 512]"""
                g = ci0 // NT
                h1 = hpool.tile([128, 2, 2, 512], F8, name=f"h1_{ci0}",
                                tag=f"h1_{ci0 % 4}", bufs=2)
                cp = (ci0 % NT) // 2
                for mt in range(2):
                    pp = ps.tile([128, 2, 512], F32, name=f"ps0_{ci0}_{mt}",
                                 tag="ps")
                    for dci in range(2):
                        nc.tensor.matmul(pp[:, dci, :],
                                         l0pack[g][:, mt, :, :],
                                         ss8c[cp][:, :, 512 * dci:512 * dci + 512],
                                         start=True, stop=True, perf_mode=DR)
                    # b0a folded via sel row 64 -> pure relu
                    evict(h1[:, mt, :, :], pp[:, :, :], None, 1024)
                return h1

            # super-pairs of 2 chunk-pairs: within each layer, pair P+1's
            # matmuls cover the eviction latency of pair P's outputs, so
            # the PE never waits at a layer boundary.
            qa_state = {}
            h1_pending = {0: emit_l0(0), 2: emit_l0(2)}
            for si in range(0, NCHUNK, 4):
                pairs = (si, si + 2)
                h_cur = {p: h1_pending.pop(p) for p in pairs}
                for wname, kt2, md in DR_LAYERS:
                    mtn = md // 128
                    for p in pairs:
                        h = h_cur[p]
                        hn = hpool.tile([128, mtn, 2, 512], F8,
                                        name=f"h_{wname}_{p}",
                                        tag=f"h_{wname}_{p % 4}", bufs=2)
                        for mt in range(mtn):
                            pp = ps.tile([128, 2, 512], F32,
                                         name=f"ps_{wname}_{p}_{mt}",
                                         tag="ps")
                            for dci in range(2):
                                for j in range(kt2):
                                    nc.tensor.matmul(
                                        pp[:, dci, :],
                                        w8[wname][:, j, :,
                                                  128 * mt:128 * (mt + 1)],
                                        h[:, 2 * j:2 * j + 2, dci, :],
                                        start=(j == 0), stop=(j == kt2 - 1),
                                        perf_mode=DR)
                            evict(hn[:, mt, :, :], pp[:, :, :],
                                  b_sb[BIAS_OF[wname]][:, mt:mt + 1], 1024)
                        h_cur[p] = hn
                # pull next super-pair's L0 here: its evictions precede
                # the tail's in the engine queues, and the tail's PE work
                # covers their latency
                if si + 4 < NCHUNK:
                    h1_pending[si + 4] = emit_l0(si + 4)
                    h1_pending[si + 6] = emit_l0(si + 6)
                h6s = {}
                for p in pairs:
                    # W2b: DR slots = the pair's two chunks of h5,
                    # zero-padded weight variants select one each
                    h5 = h_cur[p]
                    h6 = hpool.tile([128, 2, 2, 512], F8, name=f"h6_{p}",
                                    tag=f"h6_{p % 4}", bufs=2)
                    for mt in range(2):
                        pp = ps.tile([128, 2, 512], F32,
                                     name=f"ps6_{p}_{mt}", tag="ps")
                        for dci in range(2):
                            nc.tensor.matmul(
                                pp[:, dci, :],
                                w8["W2b"][:, dci, :, 128 * mt:128 * (mt + 1)],
                                h5[:, 0, :, :],
                                start=True, stop=True, perf_mode=DR)
                        evict(h6[:, mt, :, :], pp[:, :, :],
                              b_sb["b2b"][:, mt:mt + 1], 1024)
                    h6s[p] = h6
                # q_before stage: independent PE work that covers the h6
                # eviction latency before Wv reads it
                if si >= 4:
                    qb_stages[si // 4 - 1]()
                for p in pairs:
                    # Wv: K=256 DR; chunk c routes to psum partition
                    # c % 16 via its zero-padded weight variant, so a
                    # 16-chunk window accumulates in ONE [16, 512] psum
                    # bank and is evicted once (512 cols) instead of
                    # eight single-partition [1, 1024] copies.
                    for dci in range(2):
                        c = p + dci
                        w, v = c // 16, c % 16
                        if v == 0:
                            qa_state["ps"] = ps.tile(
                                [16, 512], F32, name=f"ps_qa_{w}",
                                tag="qa", bufs=2)
                        nc.tensor.matmul(qa_state["ps"][:, :],
                                         w8["Wv"][:, :, v, :],
                                         h6s[p][:, :, dci, :],
                                         start=(v == 0), stop=(v == 15),
                                         perf_mode=DR)
                        if v == 15:
                            qa_sb = qpool.tile([16, 512], F32,
                                               name=f"qa_{w}", tag="qaev",
                                               bufs=2)
                            evict_copy(qa_sb[:, :], qa_state["ps"][:, :],
                                       512)
                            nc.sync.dma_start(qa_dram[w:w + 1, :],
                                              qa_sb[:, :])
    nc.compile()
    return nc


def get_nc():
    if "nc" not in _CACHE:
        _CACHE["nc"] = _build()
    return _CACHE["nc"]


def _pack_dr(W, kt2, md):
    """[K, M] f32 -> [128, kt2*2*md] fp8 DR layout (k = (j, slot, p))."""
    W8 = W.astype(NP_F8)
    return np.ascontiguousarray(
        W8.reshape(kt2, 2, 128, md).transpose(2, 0, 1, 3).reshape(128, -1))


def make_in_maps(obs, feat, W0a, b0a, W0b, b0b, W1a, b1a, W1b, b1b,
                 W2a, b2a, W2b, b2b, Wv, bv):
    obs = np.ascontiguousarray(obs, np.float32)
    feat = np.ascontiguousarray(feat, np.float32)
    norm = np.where(np.arange(IN) % 2 == 0, 42.0, 160.0).astype(np.float32)
    nd = norm[:D]
    W0a = np.ascontiguousarray(W0a, np.float32)

    w2b8 = np.zeros((128, 2, 2, 256), NP_F8)
    w2b8[:, 0, 0, :] = np.asarray(W2b, np.float32).astype(NP_F8)
    w2b8[:, 1, 1, :] = w2b8[:, 0, 0, :]
    wv8 = np.zeros((128, 2, 16, 16), NP_F8)
    wv_col = np.asarray(Wv, np.float32).reshape(2, 128).astype(NP_F8).T
    for v in range(16):
        wv8[:, :, v, v] = wv_col
    l0w8 = np.ascontiguousarray(
        W0a.astype(NP_F8).reshape(4, 128, 2, 128).transpose(1, 0, 2, 3)
        .reshape(128, -1))
    zpad8 = np.zeros((64, 4, 256), NP_F8)
    zpad8[0, :, :] = np.asarray(b0a, np.float32).astype(NP_F8)
    zpad8 = zpad8.reshape(64, 1024)

    shared = {
        "W0a": W0a,
        "W0b": np.ascontiguousarray(W0b, np.float32),
        "W1a": np.ascontiguousarray(W1a, np.float32),
        "W1b": np.ascontiguousarray(W1b, np.float32),
        "W2a": np.ascontiguousarray(W2a, np.float32),
        "W2b": np.ascontiguousarray(W2b, np.float32),
        "Wv": np.ascontiguousarray(Wv, np.float32).reshape(256, 1),
        "W0b8": _pack_dr(np.asarray(W0b, np.float32), 1, 512),
        "W1a8": _pack_dr(np.asarray(W1a, np.float32), 2, 512),
        "W1b8": _pack_dr(np.asarray(W1b, np.float32), 2, 256),
        "W2a8": _pack_dr(np.asarray(W2a, np.float32), 1, 128),
        "W2b8": np.ascontiguousarray(w2b8.reshape(128, -1)),
        "Wv8": np.ascontiguousarray(wv8.reshape(128, -1)),
        "l0w8": l0w8,
        "zpad8": zpad8,
        "bundleF": np.ascontiguousarray(np.concatenate(
            [np.asarray(b, np.float32).reshape(ntc, 128).T
             for b, ntc in [(b0a, 2), (b0b, 4), (b1a, 4), (b1b, 2),
                            (b2a, 1), (b2b, 2)]], axis=1)),      # [128, 15]
    }
    # sel: rows 0..63 broadcast base over (i_lo, k); row 64 = ones (b0a)
    sel = np.zeros((128, 512), np.float32)
    sel[:BSL, :] = np.tile(np.eye(BSL, dtype=np.float32), (1, 512 // BSL))
    sel[64, :] = 1.0

    obs_flat = obs.reshape(BS, IN)
    in_maps = []
    for cidx in range(N_CORES):
        sl = slice(cidx * BSL, (cidx + 1) * BSL)
        obsS = (obs_flat[sl] / norm).T.reshape(4, 128, BSL)
        obsS = obsS.transpose(1, 0, 2).reshape(128, 4 * BSL)
        obsU = obs_flat[sl].T.reshape(4, 128, BSL)
        obsU = obsU.transpose(1, 0, 2).reshape(128, 4 * BSL)

        # S[4*il+d, 128*il + k*64 + b] = -feat[b, k, d] / nd[d]
        fs = -(feat[sl] / nd)                      # [64, 2, 4]
        fsT = fs.transpose(2, 1, 0).reshape(D, 2 * BSL)
        S = np.zeros((128, 4096), np.float32)
        for il in range(32):
            S[4 * il:4 * il + 4, 128 * il:128 * (il + 1)] = fsT
        ss8 = np.zeros((128, 4, 2, 1024), NP_F8)
        ss8[:, :, 0, :] = np.tile(sel, (1, 2)).astype(NP_F8)[:, None, :]
        ss8[:, :, 1, :] = S.astype(NP_F8).reshape(128, 4, 1024)

        m = dict(shared)
        m["bundleR"] = np.ascontiguousarray(obsU)                # [128, 256]
        m["obsS8"] = np.ascontiguousarray(obsS.astype(NP_F8))    # [128, 256]
        m["SS8"] = np.ascontiguousarray(ss8.reshape(128, -1))
        in_maps.append(m)
    return in_maps


def assemble(results):
    qa = np.stack([r["qa"].reshape(-1) for r in results])   # [8, 16384]
    qb = np.stack([r["qb"].reshape(-1) for r in results])   # [8, 64]
    # r = (g, i_lo, k, b) -> j = g*64 + i_lo*2 + k
    qa = qa.reshape(N_CORES, NG, 32, 2, BSL).transpose(0, 4, 1, 2, 3)
    qa = np.ascontiguousarray(qa).reshape(BS, 2 * N)
    return (qa - qb.reshape(BS, 1)).astype(np.float32)


def kernel(**inputs):
    nc = get_nc()
    in_maps = make_in_maps(**inputs)
    res = run_bass_kernel_spmd(nc, in_maps, core_ids=list(range(N_CORES)))
    return assemble(res.results)



# revision 17
# speedup vs baseline: 1.0141x; 1.0141x over previous
"""Trainium2 Bass kernel for nn_DQNAgent_modify (dense_mlp) — fp8 DoubleRow.

Reference computation:
    q_before = mlp(obs.reshape(bs, -1))                      # raw obs
    pert[b, i, k] = obs_flat[b] - onehot(i) x feat[b, k]     # bs*2N rows
    q_after = mlp(pert / norm)                               # [bs, 2N]
    out = q_after - q_before                                 # [bs, 2N]

Two structural facts drive this version:

1. Layer-0 collapse (as in the f32r baseline): the perturbation touches
   only 4 of the 512 input features, so layer 0 of the big batch is
   z = base - corr with base = (obs/norm) @ W0a computed once per
   sample; both terms are matmuls against host-built operands (sel
   broadcast matrix + block-diagonal S from feat).

2. The output q_after - q_before is norm-dominated by q_before (RMS
   ~1.67 vs q_after's ~0.033), so the big 16384-row batch tolerates
   large relative error: quantizing every big-batch matmul to fp8-e4m3
   measures 3.4e-3 final rel err (gate 2e-2).  That unlocks
   MatmulPerfMode.DoubleRow: K=256 contracted per pass at 0.5
   cycles/output-row — ~4x f32r PE throughput.  Only the 64-row
   q_before mini-MLP stays f32r.

With the PE thus accelerated the bottleneck is PSUM->SBUF relu+cast
evictions, so: psum tiles are [128, 2(chunk), 512] pairs evicted in one
op (per-mt bias stays a legal per-partition scalar), evictions rotate
across ACT/DVE/Pool by a weighted load balancer, the final Wv result is
DMA'd straight from PSUM, and b0a rides for free in the selector slot
(sel row 64 = ones, base row 64 = b0a).  W2b (K=128) uses a DoubleRow
pass whose two slots are the two chunks' h5 with zero-padded weight
variants, so it too runs at 256 cycles.

Sharding: pure data parallel over batch, 64 samples/core on 8 cores;
weights replicated.  Row order on device is r = (g, i_lo, k, b) with
i = 32g + i_lo; the host unpermutes and applies q_after - q_before.
"""

import numpy as np
import ml_dtypes

import concourse.mybir as mybir
import concourse.tile as tile
from concourse import bacc
from concourse.bass_utils import run_bass_kernel_spmd

N_CORES = 8
BS, N, D = 512, 128, 4
BSL = BS // N_CORES        # 64 samples per core
IN = N * D                 # 512 input features
NG = 4                     # i-groups == 128-row blocks of W0a
NT = 8                     # 512-row chunks per group
NCHUNK = NG * NT           # 32 chunks of 512 rows per core
F32 = mybir.dt.float32
F32R = mybir.dt.float32r
F8 = mybir.dt.float8e4
NP_F8 = ml_dtypes.float8_e4m3
DR = mybir.MatmulPerfMode.DoubleRow
RELU = mybir.ActivationFunctionType.Relu
COPY = mybir.ActivationFunctionType.Copy
ADD = mybir.AluOpType.add
MAX = mybir.AluOpType.max

# fp8 DR-packed dense layers: (name, KT2 = K/256, M)
DR_LAYERS = [("W0b", 1, 512), ("W1a", 2, 512), ("W1b", 2, 256),
             ("W2a", 1, 128)]
BIAS_OF = {"W0b": "b0b", "W1a": "b1a", "W1b": "b1b", "W2a": "b2a",
           "W2b": "b2b"}
# f32r weights for the q_before mini-MLP
WSHAPES = [("W0a", IN, 256), ("W0b", 256, 512), ("W1a", 512, 512),
           ("W1b", 512, 256), ("W2a", 256, 128), ("W2b", 128, 256),
           ("Wv", 256, 1)]
QB_LAYERS = [("W0b", 2, 4), ("W1a", 4, 4), ("W1b", 4, 2), ("W2a", 2, 1),
             ("W2b", 1, 2)]
BSHAPES = [("b0a", 2), ("b0b", 4), ("b1a", 4), ("b1b", 2), ("b2a", 1),
           ("b2b", 2)]

_CACHE = {}


def _build():
    nc = bacc.Bacc("TRN2", target_bir_lowering=False, debug=False,
                   num_devices=N_CORES)

    dram = {}
    for name, kd, md in WSHAPES:
        dram[name] = nc.dram_tensor(name, [kd, md], F32R,
                                    kind="ExternalInput").ap()
    for name, kt2, md in DR_LAYERS:
        dram[name + "8"] = nc.dram_tensor(name + "8", [128, kt2 * 2 * md],
                                          F8, kind="ExternalInput").ap()
    # W2b dual-variant: [128, (var 2, slot 2, 256)]
    dram["W2b8"] = nc.dram_tensor("W2b8", [128, 1024], F8,
                                  kind="ExternalInput").ap()
    # [128, (slot 2, var 32, 32)]: variant c has Wv in column c of its
    # 32-wide block, zeros elsewhere; 32-wide blocks keep the DR
    # ldweights slot stride 16B-aligned (s3_lw_dual_fp8_restrictions).
    # Variant c routes chunk c's Wv product to psum partition c, so all
    # 32 chunks accumulate into ONE persistent [32, 512] psum bank that
    # is evicted once at the end of the kernel.
    dram["Wv8"] = nc.dram_tensor("Wv8", [128, 2048], F8,
                                 kind="ExternalInput").ap()
    # W0a fp8 blocks for L0: [128, (g 4, mt 2, 128)]
    dram["l0w8"] = nc.dram_tensor("l0w8", [128, 1024], F8,
                                  kind="ExternalInput").ap()
    # zpad8: rows 64..127 of the base slot for all 4 g-packs
    # (row 64 = b0a, rest 0)
    dram["zpad8"] = nc.dram_tensor("zpad8", [64, 1024], F8,
                                   kind="ExternalInput").ap()
    # SS8 [128, (cpair 4, slot 2, 1024)]: per chunk PAIR, slot0 = sel
    # twice (row 64 = ones for b0a), slot1 = S columns of both chunks —
    # so one L0 DR matmul covers the pair with an AP-depth-2 ifmap.
    # On-device it lands in four per-cpair tiles: deps are tracked at
    # tile granularity, so a single tile would stall the first L0
    # matmul on ALL four DMAs.
    dram["SS8"] = nc.dram_tensor("SS8", [128, 8192], F8,
                                 kind="ExternalInput").ap()
    # obsU (raw obs for q_before, f32r)
    dram["bundleR"] = nc.dram_tensor("bundleR", [128, 256], F32R,
                                     kind="ExternalInput").ap()
    # obsS8 [128, (kt 4, 64)]: obs/norm in fp8 — base = obsS8 @ W0a runs
    # as fp8 DR so the 640KB f32r obsS/W0a DMA leaves the critical path
    dram["obsS8"] = nc.dram_tensor("obsS8", [128, 256], F8,
                                   kind="ExternalInput").ap()
    dram["bundleF"] = nc.dram_tensor("bundleF", [128, 15], F32,
                                     kind="ExternalInput").ap()
    qa_dram = nc.dram_tensor("qa", [1, NCHUNK * 512], F32,
                             kind="ExternalOutput").ap()
    qb_dram = nc.dram_tensor("qb", [1, BSL], F32, kind="ExternalOutput").ap()

    with tile.TileContext(nc) as tc:
        with (
            tc.tile_pool(name="wpool", bufs=1) as wpool,
            tc.tile_pool(name="cpool", bufs=1) as cpool,
            tc.tile_pool(name="hpool", bufs=3) as hpool,
            tc.tile_pool(name="qpool", bufs=3) as qpool,
            tc.tile_pool(name="ps", bufs=3, space="PSUM") as ps,
        ):
            # ---------------- setup DMAs ----------------
            # Every dma_start costs its issuing engine ~700ns of engine
            # time, so the ACT/DVE eviction engines issue NONE; sync and
            # gpsimd queues are ordered by first use.
            obsS8 = cpool.tile([128, 4, 64], F8, name="obsS8")
            nc.sync.dma_start(obsS8.rearrange("p k b -> p (k b)"),
                              dram["obsS8"])
            w0a8 = cpool.tile([128, 4, 2, 128], F8, name="w0a8")
            nc.sync.dma_start(w0a8.rearrange("p g mt m -> p (g mt m)"),
                              dram["l0w8"])

            # l0pack: [128, g 4, mt 2, slot 2, 128]; slot0 = base (+b0a
            # row), slot1 = W0a fp8 block — one tile, two DMA triggers
            l0pack_t = cpool.tile([128, 4, 2, 2, 128], F8, name="l0pack")
            nc.gpsimd.dma_start(
                l0pack_t[:, :, :, 1, :],
                dram["l0w8"].rearrange("p (g mt m) -> p g mt m", g=4, mt=2))
            nc.gpsimd.dma_start(
                l0pack_t[64:128, :, :, 0, :],
                dram["zpad8"].rearrange("p (g mt m) -> p g mt m", g=4, mt=2))
            l0pack = [l0pack_t[:, g, :, :, :] for g in range(4)]

            # SS8 as four per-cpair tiles: cp0/cp1 front-loaded on the
            # sync queue (needed by the two pre-loop emit_l0 calls),
            # cp2/cp3 deferred behind the fp8 weights
            ss8c = [cpool.tile([128, 2, 1024], F8, name=f"ss8_{cp}")
                    for cp in range(4)]
            for cp in (0, 1):
                nc.sync.dma_start(
                    ss8c[cp].rearrange("p s n -> p (s n)"),
                    dram["SS8"][:, 2048 * cp:2048 * (cp + 1)])

            # fp8 DR weights, spread across queues in first-use order
            w8 = {}
            w8_eng = {"W0b": nc.gpsimd, "W1a": nc.gpsimd, "W1b": nc.sync,
                      "W2a": nc.gpsimd}
            for name, kt2, md in DR_LAYERS:
                t = wpool.tile([128, kt2, 2, md], F8, name=f"w8_{name}")
                w8_eng[name].dma_start(
                    t.rearrange("p a s m -> p (a s m)"), dram[name + "8"])
                w8[name] = t
            w8["W2b"] = wpool.tile([128, 2, 2, 256], F8, name="w8_W2b")
            nc.sync.dma_start(
                w8["W2b"].rearrange("p v s m -> p (v s m)"), dram["W2b8"])
            w8["Wv"] = wpool.tile([128, 2, 32, 32], F8, name="w8_Wv")
            nc.gpsimd.dma_start(
                w8["Wv"].rearrange("p s v m -> p (s v m)"), dram["Wv8"])
            for cp, eng in ((2, nc.sync), (3, nc.gpsimd)):
                eng.dma_start(ss8c[cp].rearrange("p s n -> p (s n)"),
                              dram["SS8"][:, 2048 * cp:2048 * (cp + 1)])

            bundleF = cpool.tile([128, 15], F32, name="bundleF")
            nc.sync.dma_start(bundleF[:, :], dram["bundleF"][:, :])

            # f32r weights for q_before: 3.4 MB but not needed until the
            # first qb stage (~25 us in) — queued after all fp8 operands
            w_r = {}
            bundleR = cpool.tile([128, 256], F32R, name="bundleR")
            nc.gpsimd.dma_start(bundleR[:, :], dram["bundleR"][:, :])
            obsU_r = bundleR.rearrange("p (k b) -> p k b", k=4)
            w_r["W0a"] = wpool.tile([128, 4, 256], F32R, name="wr_W0a")
            for k in range(4):
                eng = (nc.sync, nc.gpsimd)[k % 2]
                eng.dma_start(w_r["W0a"][:, k, :],
                              dram["W0a"][128 * k:128 * (k + 1), :])
            for idx, (name, kd, md) in enumerate(WSHAPES[1:]):
                kt = kd // 128
                wr = wpool.tile([128, kt, md], F32R, name=f"wr_{name}")
                eng = (nc.gpsimd, nc.sync)[idx % 2]
                eng.dma_start(
                    wr[:, :, :],
                    dram[name].rearrange("(k p) m -> p k m", p=128))
                w_r[name] = wr

            b_off = {}
            off = 0
            for name, ntc in BSHAPES:
                b_off[name] = off
                off += ntc
            b_sb = {name: bundleF[:, b_off[name]:b_off[name] + ntc]
                    for name, ntc in BSHAPES}

            # ---- base = (obs/norm) @ W0a in fp8 DR (64 rows), evicted
            # as fp8 into slot 0 of all four l0pack tiles
            pbase = ps.tile([BSL, 256], F32, name="ps_base", tag="ps")
            for j in range(2):
                nc.tensor.matmul(
                    pbase[:, :], obsS8[:, 2 * j:2 * j + 2, :],
                    w0a8[:, 2 * j:2 * j + 2, :, :]
                    .rearrange("p k mt m -> p k (mt m)"),
                    start=(j == 0), stop=(j == 1), perf_mode=DR)
            for g in range(4):
                eng = nc.scalar if g % 2 == 0 else nc.vector
                if g % 2 == 0:
                    eng.activation(
                        l0pack[g][0:BSL, :, 0, :],
                        pbase[:, :].rearrange("b (mt m) -> b mt m", mt=2),
                        COPY)
                else:
                    eng.tensor_copy(
                        l0pack[g][0:BSL, :, 0, :],
                        pbase[:, :].rearrange("b (mt m) -> b mt m", mt=2))

            # ------------- weighted eviction scheduler -------------
            # approximate per-op engine costs (ns) incl. fixed overhead
            # GPSIMD cannot access PSUM on TRN2 -> ACT + DVE only
            ev_load = {"act": 0.0, "dve": 0.0}

            def _pick(els):
                costs = {"act": els * 0.83 + 206,
                         "dve": els * 1.04 + 137}
                eng = min(ev_load, key=lambda e: ev_load[e] + costs[e])
                ev_load[eng] += costs[eng]
                return eng

            def evict(out_ap, psum_ap, bias_ap, els):
                b = bias_ap if bias_ap is not None else 0.0
                if _pick(els) == "act":
                    nc.scalar.activation(out_ap, psum_ap, RELU, bias=b)
                else:
                    nc.vector.tensor_scalar(out_ap, psum_ap, b, 0.0, ADD, MAX)

            def evict_copy(out_ap, psum_ap, els):
                if _pick(els) == "act":
                    nc.scalar.activation(out_ap, psum_ap, COPY)
                else:
                    nc.vector.tensor_copy(out_ap, psum_ap)

            # ------- q_before mini-MLP (f32r), one layer per pair -------
            # each stage's evictions get a full pair (~9 us) to complete
            # before the next stage's matmuls need them, so the in-order
            # PE queue never blocks on the ACT/DVE queues.  All of a
            # stage's mt outputs pack into ONE 1-bank psum tile on a
            # dedicated tag, so qb stages never churn the main "ps"
            # ring (stages are ~2.4 us apart — bufs=1 never stalls).
            qb_state = {}

            def qb_stage0():
                hq = []
                pb = ps.tile([128, 4, BSL], F32, name="ps_qb0", tag="qbps",
                             bufs=1)
                for mt in range(2):
                    for kt in range(4):
                        nc.tensor.matmul(
                            pb[:, mt, :],
                            w_r["W0a"][:, kt, 128 * mt:128 * (mt + 1)],
                            obsU_r[:, kt, :],
                            start=(kt == 0), stop=(kt == 3))
                    h = qpool.tile([128, BSL], F32R, name=f"hq0_{mt}",
                                   tag=f"hq_{mt}")
                    evict(h[:, :], pb[:, mt, :], b_sb["b0a"][:, mt:mt + 1],
                          BSL)
                    hq.append(h)
                qb_state["hq"] = hq

            def qb_stage_mid(li):
                wname, ktn, mtn = QB_LAYERS[li]
                hq = qb_state.pop("hq")
                nxt = []
                pb = ps.tile([128, 4, BSL], F32, name=f"ps_qb{li+1}",
                             tag="qbps", bufs=1)
                for mt in range(mtn):
                    for kt in range(ktn):
                        nc.tensor.matmul(
                            pb[:, mt, :],
                            w_r[wname][:, kt, 128 * mt:128 * (mt + 1)],
                            hq[kt][:, :],
                            start=(kt == 0), stop=(kt == ktn - 1))
                    h = qpool.tile([128, BSL], F32R, name=f"hq{li+1}_{mt}",
                                   tag=f"hq_{mt}")
                    evict(h[:, :], pb[:, mt, :],
                          b_sb[BIAS_OF[wname]][:, mt:mt + 1], BSL)
                    nxt.append(h)
                qb_state["hq"] = nxt

            def qb_stage_out():
                hq = qb_state.pop("hq")
                pqb = ps.tile([128, 4, BSL], F32, name="ps_qb_out",
                              tag="qbps", bufs=1)
                for kt in range(2):
                    nc.tensor.matmul(pqb[0:1, 0, :], w_r["Wv"][:, kt, :],
                                     hq[kt][:, :],
                                     start=(kt == 0), stop=(kt == 1))
                qb_sb = qpool.tile([1, BSL], F32, name="qb_sb")
                evict_copy(qb_sb[:, :], pqb[0:1, 0, :], BSL)
                nc.sync.dma_start(qb_dram[:, :], qb_sb[:, :])

            qb_stages = ([qb_stage0]
                         + [lambda li=i: qb_stage_mid(li) for i in range(5)]
                         + [qb_stage_out])

            # ---------------- big-batch layers (fp8 DR) ----------------
            def emit_l0(ci0):
                """L0 for a chunk pair -> h1pair [128, mtK 2, chunk 2, 512]"""
                g = ci0 // NT
                h1 = hpool.tile([128, 2, 2, 512], F8, name=f"h1_{ci0}",
                                tag=f"h1_{ci0 % 4}", bufs=2)
                cp = (ci0 % NT) // 2
                for mt in range(2):
                    pp = ps.tile([128, 2, 512], F32, name=f"ps0_{ci0}_{mt}",
                                 tag="ps")
                    for dci in range(2):
                        nc.tensor.matmul(pp[:, dci, :],
                                         l0pack[g][:, mt, :, :],
                                         ss8c[cp][:, :, 512 * dci:512 * dci + 512],
                                         start=True, stop=True, perf_mode=DR)
                    # b0a folded via sel row 64 -> pure relu
                    evict(h1[:, mt, :, :], pp[:, :, :], None, 1024)
                return h1

            # super-pairs of 2 chunk-pairs: within each layer, pair P+1's
            # matmuls cover the eviction latency of pair P's outputs, so
            # the PE never waits at a layer boundary.
            qa_state = {}
            h1_pending = {0: emit_l0(0), 2: emit_l0(2)}
            for si in range(0, NCHUNK, 4):
                pairs = (si, si + 2)
                h_cur = {p: h1_pending.pop(p) for p in pairs}
                for wname, kt2, md in DR_LAYERS:
                    mtn = md // 128
                    for p in pairs:
                        h = h_cur[p]
                        hn = hpool.tile([128, mtn, 2, 512], F8,
                                        name=f"h_{wname}_{p}",
                                        tag=f"h_{wname}_{p % 4}", bufs=2)
                        for mt in range(mtn):
                            pp = ps.tile([128, 2, 512], F32,
                                         name=f"ps_{wname}_{p}_{mt}",
                                         tag="ps")
                            for dci in range(2):
                                for j in range(kt2):
                                    nc.tensor.matmul(
                                        pp[:, dci, :],
                                        w8[wname][:, j, :,
                                                  128 * mt:128 * (mt + 1)],
                                        h[:, 2 * j:2 * j + 2, dci, :],
                                        start=(j == 0), stop=(j == kt2 - 1),
                                        perf_mode=DR)
                            evict(hn[:, mt, :, :], pp[:, :, :],
                                  b_sb[BIAS_OF[wname]][:, mt:mt + 1], 1024)
                        h_cur[p] = hn
                # pull next super-pair's L0 here: its evictions precede
                # the tail's in the engine queues, and the tail's PE work
                # covers their latency
                if si + 4 < NCHUNK:
                    h1_pending[si + 4] = emit_l0(si + 4)
                    h1_pending[si + 6] = emit_l0(si + 6)
                h6s = {}
                for p in pairs:
                    # W2b: DR slots = the pair's two chunks of h5,
                    # zero-padded weight variants select one each
                    h5 = h_cur[p]
                    h6 = hpool.tile([128, 2, 2, 512], F8, name=f"h6_{p}",
                                    tag=f"h6_{p % 4}", bufs=2)
                    for mt in range(2):
                        pp = ps.tile([128, 2, 512], F32,
                                     name=f"ps6_{p}_{mt}", tag="ps")
                        for dci in range(2):
                            nc.tensor.matmul(
                                pp[:, dci, :],
                                w8["W2b"][:, dci, :, 128 * mt:128 * (mt + 1)],
                                h5[:, 0, :, :],
                                start=True, stop=True, perf_mode=DR)
                        evict(h6[:, mt, :, :], pp[:, :, :],
                              b_sb["b2b"][:, mt:mt + 1], 1024)
                    h6s[p] = h6
                # q_before stage: independent PE work that covers the h6
                # eviction latency before Wv reads it
                if si >= 4:
                    qb_stages[si // 4 - 1]()
                for p in pairs:
                    # Wv: K=256 DR; chunk c routes to psum partition c
                    # via its zero-padded weight variant, so all 32
                    # chunks accumulate in ONE persistent [32, 512]
                    # psum bank, evicted once (512 cols) at the end
                    # instead of sixteen single-partition [1, 1024]
                    # copies.
                    for dci in range(2):
                        c = p + dci
                        if c == 0:
                            qa_state["ps"] = ps.tile(
                                [32, 512], F32, name="ps_qa", tag="qa",
                                bufs=1)
                        nc.tensor.matmul(qa_state["ps"][:, :],
                                         w8["Wv"][:, :, c, :],
                                         h6s[p][:, :, dci, :],
                                         start=(c == 0),
                                         stop=(c == NCHUNK - 1),
                                         perf_mode=DR)
                        if c == NCHUNK - 1:
                            qa_sb = qpool.tile([32, 512], F32,
                                               name="qa_sb")
                            evict_copy(qa_sb[:, :], qa_state["ps"][:, :],
                                       512)
                            nc.sync.dma_start(qa_dram[0:1, :],
                                              qa_sb[:, :])
    nc.compile()
    return nc


def get_nc():
    if "nc" not in _CACHE:
        _CACHE["nc"] = _build()
    return _CACHE["nc"]


def _pack_dr(W, kt2, md):
    """[K, M] f32 -> [128, kt2*2*md] fp8 DR layout (k = (j, slot, p))."""
    W8 = W.astype(NP_F8)
    return np.ascontiguousarray(
        W8.reshape(kt2, 2, 128, md).transpose(2, 0, 1, 3).reshape(128, -1))


def make_in_maps(obs, feat, W0a, b0a, W0b, b0b, W1a, b1a, W1b, b1b,
                 W2a, b2a, W2b, b2b, Wv, bv):
    obs = np.ascontiguousarray(obs, np.float32)
    feat = np.ascontiguousarray(feat, np.float32)
    norm = np.where(np.arange(IN) % 2 == 0, 42.0, 160.0).astype(np.float32)
    nd = norm[:D]
    W0a = np.ascontiguousarray(W0a, np.float32)

    w2b8 = np.zeros((128, 2, 2, 256), NP_F8)
    w2b8[:, 0, 0, :] = np.asarray(W2b, np.float32).astype(NP_F8)
    w2b8[:, 1, 1, :] = w2b8[:, 0, 0, :]
    wv8 = np.zeros((128, 2, 32, 32), NP_F8)
    wv_col = np.asarray(Wv, np.float32).reshape(2, 128).astype(NP_F8).T
    for v in range(32):
        wv8[:, :, v, v] = wv_col
    l0w8 = np.ascontiguousarray(
        W0a.astype(NP_F8).reshape(4, 128, 2, 128).transpose(1, 0, 2, 3)
        .reshape(128, -1))
    zpad8 = np.zeros((64, 4, 256), NP_F8)
    zpad8[0, :, :] = np.asarray(b0a, np.float32).astype(NP_F8)
    zpad8 = zpad8.reshape(64, 1024)

    shared = {
        "W0a": W0a,
        "W0b": np.ascontiguousarray(W0b, np.float32),
        "W1a": np.ascontiguousarray(W1a, np.float32),
        "W1b": np.ascontiguousarray(W1b, np.float32),
        "W2a": np.ascontiguousarray(W2a, np.float32),
        "W2b": np.ascontiguousarray(W2b, np.float32),
        "Wv": np.ascontiguousarray(Wv, np.float32).reshape(256, 1),
        "W0b8": _pack_dr(np.asarray(W0b, np.float32), 1, 512),
        "W1a8": _pack_dr(np.asarray(W1a, np.float32), 2, 512),
        "W1b8": _pack_dr(np.asarray(W1b, np.float32), 2, 256),
        "W2a8": _pack_dr(np.asarray(W2a, np.float32), 1, 128),
        "W2b8": np.ascontiguousarray(w2b8.reshape(128, -1)),
        "Wv8": np.ascontiguousarray(wv8.reshape(128, -1)),
        "l0w8": l0w8,
        "zpad8": zpad8,
        "bundleF": np.ascontiguousarray(np.concatenate(
            [np.asarray(b, np.float32).reshape(ntc, 128).T
             for b, ntc in [(b0a, 2), (b0b, 4), (b1a, 4), (b1b, 2),
                            (b2a, 1), (b2b, 2)]], axis=1)),      # [128, 15]
    }
    # sel: rows 0..63 broadcast base over (i_lo, k); row 64 = ones (b0a)
    sel = np.zeros((128, 512), np.float32)
    sel[:BSL, :] = np.tile(np.eye(BSL, dtype=np.float32), (1, 512 // BSL))
    sel[64, :] = 1.0

    obs_flat = obs.reshape(BS, IN)
    in_maps = []
    for cidx in range(N_CORES):
        sl = slice(cidx * BSL, (cidx + 1) * BSL)
        obsS = (obs_flat[sl] / norm).T.reshape(4, 128, BSL)
        obsS = obsS.transpose(1, 0, 2).reshape(128, 4 * BSL)
        obsU = obs_flat[sl].T.reshape(4, 128, BSL)
        obsU = obsU.transpose(1, 0, 2).reshape(128, 4 * BSL)

        # S[4*il+d, 128*il + k*64 + b] = -feat[b, k, d] / nd[d]
        fs = -(feat[sl] / nd)                      # [64, 2, 4]
        fsT = fs.transpose(2, 1, 0).reshape(D, 2 * BSL)
        S = np.zeros((128, 4096), np.float32)
        for il in range(32):
            S[4 * il:4 * il + 4, 128 * il:128 * (il + 1)] = fsT
        ss8 = np.zeros((128, 4, 2, 1024), NP_F8)
        ss8[:, :, 0, :] = np.tile(sel, (1, 2)).astype(NP_F8)[:, None, :]
        ss8[:, :, 1, :] = S.astype(NP_F8).reshape(128, 4, 1024)

        m = dict(shared)
        m["bundleR"] = np.ascontiguousarray(obsU)                # [128, 256]
        m["obsS8"] = np.ascontiguousarray(obsS.astype(NP_F8))    # [128, 256]
        m["SS8"] = np.ascontiguousarray(ss8.reshape(128, -1))
        in_maps.append(m)
    return in_maps


def assemble(results):
    qa = np.stack([r["qa"].reshape(-1) for r in results])   # [8, 16384]
    qb = np.stack([r["qb"].reshape(-1) for r in results])   # [8, 64]
    # r = (g, i_lo, k, b) -> j = g*64 + i_lo*2 + k
    qa = qa.reshape(N_CORES, NG, 32, 2, BSL).transpose(0, 4, 1, 2, 3)
    qa = np.ascontiguousarray(qa).reshape(BS, 2 * N)
    return (qa - qb.reshape(BS, 1)).astype(np.float32)


def kernel(**inputs):
    nc = get_nc()
    in_maps = make_in_maps(**inputs)
    res = run_bass_kernel_spmd(nc, in_maps, core_ids=list(range(N_CORES)))
    return assemble(res.results)

